# revision 1
# baseline (speedup 1.0000x reference)
"""Longformer sliding-window self-attention on 8 Trainium2 NeuronCores.

Problem: hidden [1, 8192, 768] -> QKV projections (768x768 each) ->
12-head sliding-window attention (one-sided window 256) -> ctx [1, 8192, 768].

Sharding: sequence-parallel across 8 cores. Each core owns 1024 query
positions and recomputes K/V projections over its 1024+2*256 halo-extended
slice (host passes the transposed, zero-padded hidden slice per core).

Per-core device program (all matmuls fp32r unless noted):
  - qT [768,1024], kT [768,1536] feature-major projections (W.T @ hT).
  - v' [1536, 12*65] sequence-major projection with a ones-column per head
    (fused softmax denominator row); bias+padding handled by a K=1 matmul
    against the position-validity row.
  - Per (256-query tile x head): scores^T [768k, 256q] in PSUM via 6
    K=64 matmuls; single ACT exp -> bf16 probs; multiply by a precomputed
    0/1 band*boundary mask (DVE); 6 accumulating bf16 PV matmuls ->
    ctx' [65, 256] where row 64 is the softmax denominator.
  - Normalize in ctx' layout: reciprocal of the denominator row, broadcast
    across partitions via a K=1 matmul, multiply, and DMA out head-major
    [NT, 128, 1536]; the host transposes back to [1024, 768] per core.
"""
import numpy as np
from contextlib import ExitStack

import concourse.bass as bass
import concourse.bacc as bacc
import concourse.mybir as mybir
from concourse.tile import TileContext
from concourse.bass_utils import run_bass_kernel_spmd

F32 = mybir.dt.float32
F32R = mybir.dt.float32r
BF16 = mybir.dt.bfloat16

NCORES = 8
S, HID, H, D, W = 8192, 768, 12, 64, 256
SL = S // NCORES            # 1024 queries per core
EXT = SL + 2 * W            # 1536 extended positions (with halo)
KB = HID // 128             # 6 feature blocks
NT = SL // 256              # 4 query tiles of 256
NJ = 6                      # key tiles of 128 per query tile
NST = EXT // 128            # 12 sequence tiles for v'
EXPF = mybir.ActivationFunctionType.Exp
MUL = mybir.AluOpType.mult


def _build():
    nc = bacc.Bacc(
        "TRN2",
        target_bir_lowering=False,
        debug=False,
        num_devices=NCORES,
    )
    hT_d = nc.declare_dram_parameter("hT", [HID, EXT], F32R, isOutput=False)
    wq_d = nc.declare_dram_parameter("wq", [HID, HID], F32R, isOutput=False)
    wk_d = nc.declare_dram_parameter("wk", [HID, HID], F32R, isOutput=False)
    wv_d = nc.declare_dram_parameter("wv", [HID, HID], F32R, isOutput=False)
    bias_d = nc.declare_dram_parameter("biasqk", [128, 2 * KB], F32, isOutput=False)
    pvt_d = nc.declare_dram_parameter("pvt", [128, NST], F32, isOutput=False)
    pvrow_d = nc.declare_dram_parameter("pvrow", [1, EXT], F32R, isOutput=False)
    bvrow_d = nc.declare_dram_parameter("bvrow", [1, HID], F32R, isOutput=False)
    sel2_d = nc.declare_dram_parameter("sel2", [1, 256], F32R, isOutput=False)
    out_d = nc.declare_dram_parameter("out", [NT, 128, NJ * 256], F32, isOutput=True)

    with ExitStack() as ctx:
        tc = ctx.enter_context(TileContext(nc))
        pH = ctx.enter_context(tc.tile_pool(name="h", bufs=1))
        pW = ctx.enter_context(tc.tile_pool(name="w", bufs=12))
        pQ = ctx.enter_context(tc.tile_pool(name="q", bufs=1))
        pK = ctx.enter_context(tc.tile_pool(name="k", bufs=1))
        pV = ctx.enter_context(tc.tile_pool(name="v", bufs=1))
        pProb = ctx.enter_context(tc.tile_pool(name="prob", bufs=2))
        pMask = ctx.enter_context(tc.tile_pool(name="mask", bufs=1))
        pOut = ctx.enter_context(tc.tile_pool(name="outp", bufs=2))
        pMisc = ctx.enter_context(tc.tile_pool(name="misc", bufs=1))
        pBc = ctx.enter_context(tc.tile_pool(name="bc", bufs=2))
        pSc = ctx.enter_context(tc.tile_pool(name="scps", bufs=2, space="PSUM"))
        pPs = ctx.enter_context(tc.tile_pool(name="ps", bufs=2, space="PSUM"))

        # ---- constants / small inputs
        bias_sb = pMisc.tile([128, 2 * KB], F32, tag="bias")
        nc.sync.dma_start(bias_sb[:], bias_d[:])
        pvt_sb = pMisc.tile([128, NST], F32, tag="pvt")
        nc.sync.dma_start(pvt_sb[:], pvt_d[:])
        pvrow_sb = pMisc.tile([1, EXT], F32R, tag="pvrow")
        nc.sync.dma_start(pvrow_sb[:], pvrow_d[:])
        bvrow_sb = pMisc.tile([1, HID], F32R, tag="bvrow")
        nc.sync.dma_start(bvrow_sb[:], bvrow_d[:])
        sel2_sb = pMisc.tile([1, 256], F32R, tag="sel2")
        nc.sync.dma_start(sel2_sb[:], sel2_d[:])

        # per-q-tile 0/1 masks in scores^T layout [k-part, (j, c)]:
        # band: valid iff 0 <= p + 128*j - c <= 2*W; then multiply by the
        # sequence-boundary validity of each key position (broadcast over c).
        masks = []
        for t in range(NT):
            mk = pMask.tile([128, NJ * 256], BF16, tag=f"m{t}", name=f"mask{t}")
            nc.gpsimd.memset(mk[:], 1.0)
            nc.gpsimd.affine_select(
                out=mk[:], in_=mk[:], compare_op=mybir.AluOpType.is_ge,
                fill=0.0, base=0, pattern=[[128, NJ], [-1, 256]],
                channel_multiplier=1)
            nc.gpsimd.affine_select(
                out=mk[:], in_=mk[:], compare_op=mybir.AluOpType.is_ge,
                fill=0.0, base=2 * W, pattern=[[-128, NJ], [1, 256]],
                channel_multiplier=-1)
            mv = mk[:].rearrange("p (j c) -> p j c", j=NJ)
            pvv = pvt_sb[:, 2 * t: 2 * t + NJ].rearrange("p (j c) -> p j c", c=1)
            _, pvb = bass.broadcast_tensor_aps(mv, pvv)
            nc.vector.tensor_tensor(mv, mv, pvb, MUL)
            masks.append(mk)

        # ---- weight & hidden DMAs (row-tiles of 128 partitions), ordered so
        # the first q-projection matmuls can start as early as possible
        wq_t = []
        h_t = [[None] * 3 for _ in range(KB)]
        for k in range(KB):
            w = pW.tile([128, HID], F32R, tag="w")
            nc.sync.dma_start(w[:], wq_d[k * 128:(k + 1) * 128, :])
            wq_t.append(w)
            ht = pH.tile([128, 512], F32R, tag=f"h{k}_0")
            nc.sync.dma_start(ht[:], hT_d[k * 128:(k + 1) * 128, 0:512])
            h_t[k][0] = ht
        for c in (1, 2):
            for k in range(KB):
                ht = pH.tile([128, 512], F32R, tag=f"h{k}_{c}")
                nc.sync.dma_start(ht[:], hT_d[k * 128:(k + 1) * 128,
                                              c * 512:(c + 1) * 512])
                h_t[k][c] = ht
        wk_t = []
        for k in range(KB):
            w = pW.tile([128, HID], F32R, tag="w")
            nc.sync.dma_start(w[:], wk_d[k * 128:(k + 1) * 128, :])
            wk_t.append(w)

        qT_t = [pQ.tile([128, SL], F32R, tag=f"q{m}", name=f"qT{m}") for m in range(KB)]
        kT_t = [pK.tile([128, EXT], F32R, tag=f"k{m}", name=f"kT{m}") for m in range(KB)]
        v_t = [pV.tile([128, H * 65], BF16, tag=f"v{st}", name=f"vp{st}") for st in range(NST)]

        def emit_qproj(c4):
            eo = W + c4 * 256
            ch, off = eo // 512, eo % 512
            for m in range(KB):
                ps = pPs.tile([128, 256], F32, tag="ps")
                for k in range(KB):
                    nc.tensor.matmul(
                        ps[:], lhsT=wq_t[k][:, m * 128:(m + 1) * 128],
                        rhs=h_t[k][ch][:, off:off + 256],
                        start=(k == 0), stop=(k == KB - 1))
                nc.vector.tensor_scalar_add(
                    qT_t[m][:, c4 * 256:(c4 + 1) * 256], ps[:],
                    bias_sb[:, m:m + 1])

        def emit_kproj(c):
            for m in range(KB):
                ps = pPs.tile([128, 512], F32, tag="ps")
                for k in range(KB):
                    nc.tensor.matmul(
                        ps[:], lhsT=wk_t[k][:, m * 128:(m + 1) * 128],
                        rhs=h_t[k][c][:, :],
                        start=(k == 0), stop=(k == KB - 1))
                nc.vector.tensor_scalar_add(
                    kT_t[m][:, c * 512:(c + 1) * 512], ps[:],
                    bias_sb[:, KB + m:KB + m + 1])

        def emit_vproj(st):
            vt = v_t[st]
            vv = vt[:].rearrange("p (h x) -> p h x", x=65)
            nc.gpsimd.memset(vv[:, :, 64:65], 1.0)
            ch, off = st // 4, (st % 4) * 128
            for (f0, nf) in ((0, 512), (512, 256)):
                ps = pPs.tile([128, nf], F32, tag="ps")
                for k in range(KB):
                    nc.tensor.matmul(
                        ps[:], lhsT=h_t[k][ch][:, off:off + 128],
                        rhs=wv_t[k][:, f0:f0 + nf],
                        start=(k == 0), stop=False)
                nc.tensor.matmul(
                    ps[:], lhsT=pvrow_sb[0:1, st * 128:(st + 1) * 128],
                    rhs=bvrow_sb[0:1, f0:f0 + nf], start=False, stop=True)
                nc.vector.tensor_copy(
                    vv[:, f0 // 64:(f0 + nf) // 64, 0:64],
                    ps[:].rearrange("p (h x) -> p h x", x=64))

        def emit_scores(t, h):
            kb, po = h // 2, (h % 2) * 64
            sc = pSc.tile([128, NJ * 256], F32, tag="sc")
            for j in range(NJ):
                k0 = t * 256 + j * 128
                nc.tensor.matmul(
                    sc[:, j * 256:(j + 1) * 256],
                    lhsT=kT_t[kb][po:po + 64, k0:k0 + 128],
                    rhs=qT_t[kb][po:po + 64, t * 256:(t + 1) * 256],
                    start=True, stop=True)
            pr = pProb.tile([128, NJ * 256], BF16, tag="pr")
            nc.scalar.activation(pr[:], sc[:], EXPF)
            nc.vector.tensor_mul(pr[:], pr[:], masks[t][:])
            return pr

        def emit_pv(t, h, prm, ob, rr):
            cx = pPs.tile([65, 256], F32, tag="ps")
            for j in range(NJ):
                nc.tensor.matmul(
                    cx[:], lhsT=v_t[2 * t + j][:, h * 65:(h + 1) * 65],
                    rhs=prm[:, j * 256:(j + 1) * 256],
                    start=(j == 0), stop=(j == NJ - 1))
            # stash unnormalized ctx' and the denominator reciprocal; the
            # actual normalization is batched at the end of the q-tile so
            # the PE stream stays dense (no per-head PE<->DVE ping-pong).
            c0 = (h // 2) * 256
            nc.vector.tensor_copy(
                ob[(h % 2) * 64:(h % 2) * 64 + 64, c0:c0 + 256], cx[0:64, :])
            with nc.allow_low_precision(reason="f32r has full fp32 bits"):
                nc.vector.reciprocal(rr[h % 2][0:1, c0:c0 + 256], cx[64:65, :])

        def emit_attn(t, warm=()):
            # warm: projection emitters injected after the first two score
            # groups -- dense PE filler while the first exp/mask round-trips,
            # so the PE never idles long enough for HAM to re-throttle.
            LOOK = 2
            ob = pOut.tile([128, NJ * 256], F32, tag="out", name=f"ob{t}")
            rr = [pBc.tile([1, NJ * 256], F32R, tag="rr0", name=f"rr0_{t}", bufs=1),
                  pBc.tile([1, NJ * 256], F32R, tag="rr1", name=f"rr1_{t}", bufs=1)]
            prs = {}
            for i in range(H + LOOK):
                if i < H:
                    prs[i] = emit_scores(t, i)
                if i == 1:
                    for fn in warm:
                        fn()
                if i >= LOOK:
                    emit_pv(t, i - LOOK, prs.pop(i - LOOK), ob, rr)
            return ob, rr

        def emit_norm(t, ob, rr):
            # normalize: ob[p, c] *= rr[p // 64][c] via K=1 selector matmuls
            # that broadcast the reciprocal rows across partitions
            for ck in range(3):
                bc = pPs.tile([128, 512], F32, tag="ps")
                nc.tensor.matmul(
                    bc[:], lhsT=sel2_sb[0:1, 0:128],
                    rhs=rr[0][0:1, ck * 512:(ck + 1) * 512],
                    start=True, stop=False)
                nc.tensor.matmul(
                    bc[:], lhsT=sel2_sb[0:1, 128:256],
                    rhs=rr[1][0:1, ck * 512:(ck + 1) * 512],
                    start=False, stop=True)
                nc.vector.tensor_tensor(
                    ob[:, ck * 512:(ck + 1) * 512],
                    ob[:, ck * 512:(ck + 1) * 512], bc[:], MUL)
            nc.sync.dma_start(out_d[t, :, :], ob[:])

        # ---- schedule: early slices first so attention overlaps
        # projections; remaining projection work is woven into the attention
        # pipeline as PE filler (HAM anti-throttle), and each tile's
        # normalization is deferred behind the next dense block.
        for c4 in range(NT):
            emit_qproj(c4)
        emit_kproj(0)
        emit_kproj(1)
        wv_t = []
        for k in range(KB):
            w = pW.tile([128, HID], F32R, tag="w")
            nc.sync.dma_start(w[:], wv_d[k * 128:(k + 1) * 128, :])
            wv_t.append(w)
        for st in range(4):
            emit_vproj(st)
        ob0, rr0 = emit_attn(0, warm=(lambda: emit_vproj(4), lambda: emit_vproj(5)))
        emit_kproj(2)
        emit_norm(0, ob0, rr0)
        ob1, rr1 = emit_attn(1, warm=(lambda: emit_vproj(6), lambda: emit_vproj(7)))
        emit_vproj(8)
        emit_norm(1, ob1, rr1)
        ob2, rr2 = emit_attn(2, warm=(lambda: emit_vproj(9),))
        emit_vproj(10)
        emit_norm(2, ob2, rr2)
        ob3, rr3 = emit_attn(3, warm=(lambda: emit_vproj(11),))
        emit_norm(3, ob3, rr3)

    nc.compile()
    return nc


_NC = None


def _get_nc():
    global _NC
    if _NC is None:
        _NC = _build()
    return _NC


def _prepare_in_maps(hidden_states, Wq, bq, Wk, bk, Wv, bv):
    hidden_states = np.asarray(hidden_states, dtype=np.float32)
    Wq = np.asarray(Wq, dtype=np.float32)
    Wk = np.asarray(Wk, dtype=np.float32)
    Wv = np.asarray(Wv, dtype=np.float32)
    bq = np.asarray(bq, dtype=np.float32)
    bk = np.asarray(bk, dtype=np.float32)
    bv = np.asarray(bv, dtype=np.float32)

    scale = 1.0 / np.sqrt(D).astype(np.float32)
    hT = np.ascontiguousarray(hidden_states.reshape(S, HID).T)  # [768, 8192]
    wq_s = np.ascontiguousarray(Wq * scale)
    biasqk = np.concatenate(
        [(bq * scale).reshape(KB, 128).T, bk.reshape(KB, 128).T], axis=1)
    biasqk = np.ascontiguousarray(biasqk, dtype=np.float32)
    bvrow = np.ascontiguousarray(bv.reshape(1, HID))

    in_maps = []
    for c in range(NCORES):
        lo, hi = c * SL - W, c * SL + SL + W
        padl, padr = max(0, -lo), max(0, hi - S)
        hT_c = np.zeros((HID, EXT), dtype=np.float32)
        hT_c[:, padl:EXT - padr] = hT[:, lo + padl:hi - padr]
        pv = np.zeros(EXT, dtype=np.float32)
        pv[padl:EXT - padr] = 1.0
        sel2 = (np.arange(128)[None, :] // 64 == np.arange(2)[:, None]).reshape(1, 256)
        in_maps.append(dict(
            sel2=np.ascontiguousarray(sel2.astype(np.float32)),
            hT=hT_c,
            wq=wq_s, wk=Wk, wv=Wv,
            biasqk=biasqk,
            pvt=np.ascontiguousarray(pv.reshape(NST, 128).T),
            pvrow=np.ascontiguousarray(pv.reshape(1, EXT)),
            bvrow=bvrow,
        ))
    return in_maps


def kernel(hidden_states, Wq, bq, Wk, bk, Wv, bv):
    nc = _get_nc()
    in_maps = _prepare_in_maps(hidden_states, Wq, bq, Wk, bk, Wv, bv)
    res = run_bass_kernel_spmd(nc, in_maps, list(range(NCORES)))
    out = np.empty((NCORES, SL, HID), dtype=np.float32)
    for c in range(NCORES):
        raw = res.results[c]["out"]              # [NT, 128, 1536]
        blk = raw.reshape(NT, 2, 64, NJ, 256)    # [t, hrow, d, hcol, q]
        # head h = hcol*2 + hrow, ctx[t*256+q, h, d]
        out[c] = blk.transpose(0, 4, 3, 1, 2).reshape(SL, HID)
    return out.reshape(1, S, HID)



# revision 11
# speedup vs baseline: 1.4563x; 1.4563x over previous
"""Longformer sliding-window self-attention on 8 Trainium2 NeuronCores.

Problem: hidden [1, 8192, 768] -> QKV projections (768x768 each) ->
12-head sliding-window attention (one-sided window 256) -> ctx [1, 8192, 768].

Sharding: sequence-parallel across 8 cores. Each core owns 1024 query
positions and recomputes K/V projections over its 1024+2*256 halo-extended
slice (host passes the transposed, zero-padded hidden slice per core).

v2 (this file): all matmuls bf16 (hidden/W/q/k/v/probs), which halves
weight-load (LDWEIGHTS) time and input DMA vs the fp32r baseline and keeps
the PE stream dense so the HAM clock gate stays at 8/8. Sequence-boundary
masking is folded into the v' validity column (col 64 per head) and the
zeroed v rows, so only the band-geometry mask remains: a single static
[128, 1024] bf16 mask applied to the 4 of 6 key blocks that intersect the
band edge (score columns are ordered (j0,j1,j4,j5,j2,j3) so one DVE
multiply covers them). Softmax denominators are extracted from PV row 64
by ACT Copy into a [2, 1536] row pair per q-tile and inverted with one
reciprocal_approx_fast, replacing 48 single-partition DVE reciprocals.

Per-core device program:
  - qT [768,1024], kT [768,1536] feature-major bf16 projections.
  - v' [1536, 12*65] bf16 with per-head validity column (softmax
    denominator row); bias+padding via a K=1 matmul against the
    position-validity row.
  - Per (256-query tile x head): scores^T [768k, 256q] in PSUM via 6
    K=64 bf16 matmuls; ACT exp -> bf16 probs; band-mask multiply on the
    first 1024 cols; 6 accumulating bf16 PV matmuls -> ctx' [65, 256].
  - Normalize: ACT-copied denominators -> reciprocal_approx_fast ->
    K=2 selector matmul broadcast -> DVE multiply -> DMA out.
"""
import numpy as np
from contextlib import ExitStack

import ml_dtypes

import concourse.bass as bass
import concourse.bacc as bacc
import concourse.mybir as mybir
from concourse.tile import TileContext
from concourse.bass_utils import run_bass_kernel_spmd

F32 = mybir.dt.float32
F32R = mybir.dt.float32r
BF16 = mybir.dt.bfloat16

NCORES = 8
S, HID, H, D, W = 8192, 768, 12, 64, 256
SL = S // NCORES            # 1024 queries per core
EXT = SL + 2 * W            # 1536 extended positions (with halo)
KB = HID // 128             # 6 feature blocks
NT = SL // 256              # 4 query tiles of 256
NJ = 6                      # key tiles of 128 per query tile
NST = EXT // 128            # 12 sequence tiles for v'
EXPF = mybir.ActivationFunctionType.Exp
COPYF = mybir.ActivationFunctionType.Copy
MUL = mybir.AluOpType.mult
# score column slot per key block j: (j0,j1,j4,j5,j2,j3) so the 4 blocks
# that need the band mask are contiguous in cols [0, 1024)
COL = {0: 0, 1: 1, 4: 2, 5: 3, 2: 4, 3: 5}


def _build():
    nc = bacc.Bacc(
        "TRN2",
        target_bir_lowering=False,
        debug=False,
        num_devices=NCORES,
    )
    hT_d = nc.declare_dram_parameter("hT", [HID, EXT], BF16, isOutput=False)
    wq_d = nc.declare_dram_parameter("wq", [HID, HID], BF16, isOutput=False)
    wk_d = nc.declare_dram_parameter("wk", [HID, HID], BF16, isOutput=False)
    wv_d = nc.declare_dram_parameter("wv", [HID, HID], BF16, isOutput=False)
    bias_d = nc.declare_dram_parameter("biasqk", [128, 2 * KB], F32, isOutput=False)
    pvt_d = nc.declare_dram_parameter("pvt", [128, NST], F32, isOutput=False)
    pvrow_d = nc.declare_dram_parameter("pvrow", [1, EXT], BF16, isOutput=False)
    bvrow_d = nc.declare_dram_parameter("bvrow", [1, HID], BF16, isOutput=False)
    sel2_d = nc.declare_dram_parameter("sel2", [1, 256], F32R, isOutput=False)
    out_d = nc.declare_dram_parameter("out", [NT, 128, NJ * 256], F32, isOutput=True)

    with ExitStack() as ctx:
        tc = ctx.enter_context(TileContext(nc))
        pH = ctx.enter_context(tc.tile_pool(name="h", bufs=1))
        pW = ctx.enter_context(tc.tile_pool(name="w", bufs=12))
        pQ = ctx.enter_context(tc.tile_pool(name="q", bufs=1))
        pK = ctx.enter_context(tc.tile_pool(name="k", bufs=1))
        pV = ctx.enter_context(tc.tile_pool(name="v", bufs=1))
        pProb = ctx.enter_context(tc.tile_pool(name="prob", bufs=3))
        pMask = ctx.enter_context(tc.tile_pool(name="mask", bufs=1))
        pOut = ctx.enter_context(tc.tile_pool(name="outp", bufs=2))
        pRr = ctx.enter_context(tc.tile_pool(name="rr", bufs=2))
        pMisc = ctx.enter_context(tc.tile_pool(name="misc", bufs=1))
        pSc = ctx.enter_context(tc.tile_pool(name="scps", bufs=2, space="PSUM"))
        pPs = ctx.enter_context(tc.tile_pool(name="ps", bufs=2, space="PSUM"))

        # ---- constants / small inputs
        bias_sb = pMisc.tile([128, 2 * KB], F32, tag="bias")
        nc.sync.dma_start(bias_sb[:], bias_d[:])
        pvt_sb = pMisc.tile([128, NST], F32, tag="pvt")
        nc.sync.dma_start(pvt_sb[:], pvt_d[:])
        pvrow_sb = pMisc.tile([1, EXT], BF16, tag="pvrow")
        nc.sync.dma_start(pvrow_sb[:], pvrow_d[:])
        bvrow_sb = pMisc.tile([1, HID], BF16, tag="bvrow")
        nc.sync.dma_start(bvrow_sb[:], bvrow_d[:])
        sel2_sb = pMisc.tile([1, 256], F32R, tag="sel2")
        nc.sync.dma_start(sel2_sb[:], sel2_d[:])

        # static band mask in scores^T layout [k-part, (slot, c)] for the 4
        # key blocks (j=0,1,4,5) that intersect the band edge:
        #   j0/j1 (slots 0,1): valid iff p + 128*j - c >= 0
        #   j4/j5 (slots 2,3): valid iff p + 128*(j-4) - c <= 0
        mk = pMask.tile([128, 4 * 256], BF16, tag="mask", name="mask")
        nc.gpsimd.memset(mk[:], 1.0)
        nc.gpsimd.affine_select(
            out=mk[:, 0:512], in_=mk[:, 0:512], compare_op=mybir.AluOpType.is_ge,
            fill=0.0, base=0, pattern=[[128, 2], [-1, 256]],
            channel_multiplier=1)
        nc.gpsimd.affine_select(
            out=mk[:, 512:1024], in_=mk[:, 512:1024],
            compare_op=mybir.AluOpType.is_ge,
            fill=0.0, base=0, pattern=[[-128, 2], [1, 256]],
            channel_multiplier=-1)

        # ---- weight & hidden DMAs, ordered so q-projection starts ASAP
        wq_t = []
        h_t = []
        for k in range(KB):
            w = pW.tile([128, HID], BF16, tag="w")
            nc.sync.dma_start(w[:], wq_d[k * 128:(k + 1) * 128, :])
            wq_t.append(w)
            ht = pH.tile([128, EXT], BF16, tag=f"h{k}")
            nc.sync.dma_start(ht[:], hT_d[k * 128:(k + 1) * 128, :])
            h_t.append(ht)
        wk_t = []
        for k in range(KB):
            w = pW.tile([128, HID], BF16, tag="w")
            nc.sync.dma_start(w[:], wk_d[k * 128:(k + 1) * 128, :])
            wk_t.append(w)

        qT_t = [pQ.tile([128, SL], BF16, tag=f"q{m}", name=f"qT{m}") for m in range(KB)]
        kT_t = [pK.tile([128, EXT], BF16, tag=f"k{m}", name=f"kT{m}") for m in range(KB)]
        v_t = [pV.tile([128, H * 65], BF16, tag=f"v{st}", name=f"vp{st}") for st in range(NST)]

        def emit_qproj(c2):
            e0 = W + c2 * 512
            for m in range(KB):
                ps = pPs.tile([128, 512], F32, tag="ps")
                for k in range(KB):
                    nc.tensor.matmul(
                        ps[:], lhsT=wq_t[k][:, m * 128:(m + 1) * 128],
                        rhs=h_t[k][:, e0:e0 + 512],
                        start=(k == 0), stop=(k == KB - 1))
                nc.vector.tensor_scalar_add(
                    qT_t[m][:, c2 * 512:(c2 + 1) * 512], ps[:],
                    bias_sb[:, m:m + 1])

        def emit_kproj_m(c, m):
            ps = pPs.tile([128, 512], F32, tag="ps")
            for k in range(KB):
                nc.tensor.matmul(
                    ps[:], lhsT=wk_t[k][:, m * 128:(m + 1) * 128],
                    rhs=h_t[k][:, c * 512:(c + 1) * 512],
                    start=(k == 0), stop=(k == KB - 1))
            nc.vector.tensor_scalar_add(
                kT_t[m][:, c * 512:(c + 1) * 512], ps[:],
                bias_sb[:, KB + m:KB + m + 1])

        def emit_kproj(c):
            for m in range(KB):
                emit_kproj_m(c, m)

        def emit_vproj(st):
            vt = v_t[st]
            vv = vt[:].rearrange("p (h x) -> p h x", x=65)
            # validity column doubles as the softmax-denominator selector:
            # invalid (zero-padded) key rows contribute to neither ctx nor
            # the denominator, which subsumes the sequence-boundary mask
            dcol = vv[:, :, 64:65]
            pvv = pvt_sb[:, st:st + 1].rearrange("p (a b) -> p a b", a=1)
            _, pvb = bass.broadcast_tensor_aps(dcol, pvv)
            nc.vector.tensor_copy(dcol, pvb)
            for (f0, nf) in ((0, 512), (512, 256)):
                ps = pPs.tile([128, nf], F32, tag="ps")
                for k in range(KB):
                    nc.tensor.matmul(
                        ps[:], lhsT=h_t[k][:, st * 128:(st + 1) * 128],
                        rhs=wv_t[k][:, f0:f0 + nf],
                        start=(k == 0), stop=False)
                nc.tensor.matmul(
                    ps[:], lhsT=pvrow_sb[0:1, st * 128:(st + 1) * 128],
                    rhs=bvrow_sb[0:1, f0:f0 + nf], start=False, stop=True)
                nc.vector.tensor_copy(
                    vv[:, f0 // 64:(f0 + nf) // 64, 0:64],
                    ps[:].rearrange("p (h x) -> p h x", x=64))

        def emit_scores(t, h):
            kb, po = h // 2, (h % 2) * 64
            sc = pSc.tile([128, NJ * 256], F32, tag="sc")
            for j in range(NJ):
                k0 = t * 256 + j * 128
                cs = COL[j] * 256
                nc.tensor.matmul(
                    sc[:, cs:cs + 256],
                    lhsT=kT_t[kb][po:po + 64, k0:k0 + 128],
                    rhs=qT_t[kb][po:po + 64, t * 256:(t + 1) * 256],
                    start=True, stop=True)
            pr = pProb.tile([128, NJ * 256], BF16, tag="pr")
            nc.scalar.activation(pr[:], sc[:], EXPF)
            nc.vector.tensor_mul(pr[:, 0:1024], pr[:, 0:1024], mk[:])
            return pr

        def emit_pv(t, h, prm, ob, rr):
            cx = pPs.tile([65, 256], F32, tag="ps")
            for j in range(NJ):
                cs = COL[j] * 256
                nc.tensor.matmul(
                    cx[:], lhsT=v_t[2 * t + j][:, h * 65:(h + 1) * 65],
                    rhs=prm[:, cs:cs + 256],
                    start=(j == 0), stop=(j == NJ - 1))
            # stash unnormalized ctx' and the denominator row; normalization
            # is batched per q-tile so the PE stream stays dense
            c0 = (h // 2) * 256
            nc.vector.tensor_copy(
                ob[(h % 2) * 64:(h % 2) * 64 + 64, c0:c0 + 256], cx[0:64, :])
            nc.scalar.activation(
                rr[h % 2][0:1, c0:c0 + 256], cx[64:65, :], COPYF)

        def emit_attn(t, warm=()):
            # warm: projection emitters woven in as dense PE filler so the
            # HAM clock gate never sees an idle window
            LOOK = 2
            warm = list(warm)
            ob = pOut.tile([128, NJ * 256], F32, tag="out", name=f"ob{t}")
            rr = [pRr.tile([1, NJ * 256], F32R, tag="rr0", name=f"rr0_{t}"),
                  pRr.tile([1, NJ * 256], F32R, tag="rr1", name=f"rr1_{t}")]
            prs = {}
            for i in range(H + LOOK):
                if i < H:
                    prs[i] = emit_scores(t, i)
                if i % 2 == 1 and warm:
                    warm.pop(0)()
                if i >= LOOK:
                    emit_pv(t, i - LOOK, prs.pop(i - LOOK), ob, rr)
            while warm:
                warm.pop(0)()
            return ob, rr

        def emit_norm(t, ob, rr):
            # broadcast the RAW denominators across partitions with K=1
            # selector matmuls, then invert the wide [128, 512] tile (fast:
            # 512 elems/lane) and multiply — avoids single-partition
            # reciprocals entirely
            for ck in range(3):
                bc = pPs.tile([128, 512], F32, tag="ps")
                nc.tensor.matmul(
                    bc[:], lhsT=sel2_sb[0:1, 0:128],
                    rhs=rr[0][0:1, ck * 512:(ck + 1) * 512],
                    start=True, stop=False)
                nc.tensor.matmul(
                    bc[:], lhsT=sel2_sb[0:1, 128:256],
                    rhs=rr[1][0:1, ck * 512:(ck + 1) * 512],
                    start=False, stop=True)
                bci = pRr.tile([128, 512], F32, tag="bci", name=f"bci{t}_{ck}")
                nc.vector.reciprocal_approx_fast(bci[:], bc[:])
                nc.vector.tensor_tensor(
                    ob[:, ck * 512:(ck + 1) * 512],
                    ob[:, ck * 512:(ck + 1) * 512], bci[:], MUL)
            nc.sync.dma_start(out_d[t, :, :], ob[:])

        # ---- schedule: early slices first so attention overlaps
        # projections; remaining projection work is woven into the attention
        # pipeline as PE filler, and each tile's normalization is deferred
        # behind the next dense block.
        emit_qproj(0)
        emit_qproj(1)
        emit_kproj(0)
        emit_kproj(1)
        wv_t = []
        for k in range(KB):
            w = pW.tile([128, HID], BF16, tag="w")
            nc.sync.dma_start(w[:], wv_d[k * 128:(k + 1) * 128, :])
            wv_t.append(w)
        for st in range(6):
            emit_vproj(st)
        ob0, rr0 = emit_attn(0, warm=(
            lambda: emit_vproj(6), lambda: emit_vproj(7),
            lambda: emit_kproj_m(2, 0), lambda: emit_kproj_m(2, 1),
            lambda: emit_kproj_m(2, 2)))
        emit_norm(0, ob0, rr0)
        ob1, rr1 = emit_attn(1, warm=(
            lambda: emit_kproj_m(2, 3), lambda: emit_kproj_m(2, 4),
            lambda: emit_kproj_m(2, 5),
            lambda: emit_vproj(8), lambda: emit_vproj(9)))
        emit_norm(1, ob1, rr1)
        ob2, rr2 = emit_attn(2, warm=(
            lambda: emit_vproj(10), lambda: emit_vproj(11)))
        emit_norm(2, ob2, rr2)
        ob3, rr3 = emit_attn(3)
        emit_norm(3, ob3, rr3)

    nc.compile()
    return nc


_NC = None


def _get_nc():
    global _NC
    if _NC is None:
        _NC = _build()
    return _NC


def _bf16(x):
    return np.ascontiguousarray(x.astype(ml_dtypes.bfloat16))


def _prepare_in_maps(hidden_states, Wq, bq, Wk, bk, Wv, bv):
    hidden_states = np.asarray(hidden_states, dtype=np.float32)
    Wq = np.asarray(Wq, dtype=np.float32)
    Wk = np.asarray(Wk, dtype=np.float32)
    Wv = np.asarray(Wv, dtype=np.float32)
    bq = np.asarray(bq, dtype=np.float32)
    bk = np.asarray(bk, dtype=np.float32)
    bv = np.asarray(bv, dtype=np.float32)

    scale = 1.0 / np.sqrt(D).astype(np.float32)
    hT = np.ascontiguousarray(hidden_states.reshape(S, HID).T)  # [768, 8192]
    wq_b = _bf16(Wq * scale)
    wk_b = _bf16(Wk)
    wv_b = _bf16(Wv)
    biasqk = np.concatenate(
        [(bq * scale).reshape(KB, 128).T, bk.reshape(KB, 128).T], axis=1)
    biasqk = np.ascontiguousarray(biasqk, dtype=np.float32)
    bvrow = _bf16(bv.reshape(1, HID))
    sel2 = (np.arange(128)[None, :] // 64 == np.arange(2)[:, None]).reshape(1, 256)
    sel2 = np.ascontiguousarray(sel2.astype(np.float32))

    in_maps = []
    for c in range(NCORES):
        lo, hi = c * SL - W, c * SL + SL + W
        padl, padr = max(0, -lo), max(0, hi - S)
        hT_c = np.zeros((HID, EXT), dtype=np.float32)
        hT_c[:, padl:EXT - padr] = hT[:, lo + padl:hi - padr]
        pv = np.zeros(EXT, dtype=np.float32)
        pv[padl:EXT - padr] = 1.0
        in_maps.append(dict(
            sel2=sel2,
            hT=_bf16(hT_c),
            wq=wq_b, wk=wk_b, wv=wv_b,
            biasqk=biasqk,
            pvt=np.ascontiguousarray(pv.reshape(NST, 128).T),
            pvrow=_bf16(pv.reshape(1, EXT)),
            bvrow=bvrow,
        ))
    return in_maps


def kernel(hidden_states, Wq, bq, Wk, bk, Wv, bv):
    nc = _get_nc()
    in_maps = _prepare_in_maps(hidden_states, Wq, bq, Wk, bk, Wv, bv)
    res = run_bass_kernel_spmd(nc, in_maps, list(range(NCORES)))
    out = np.empty((NCORES, SL, HID), dtype=np.float32)
    for c in range(NCORES):
        raw = res.results[c]["out"]              # [NT, 128, 1536]
        blk = raw.reshape(NT, 2, 64, NJ, 256)    # [t, hrow, d, hcol, q]
        # head h = hcol*2 + hrow, ctx[t*256+q, h, d]
        out[c] = blk.transpose(0, 4, 3, 1, 2).reshape(SL, HID)
    return out.reshape(1, S, HID)
